# revision 1
# baseline (speedup 1.0000x reference)
"""Distributed multi-head attention for TRN2 (8 NeuronCores).

Reference computation (per problem spec):
    q = (query @ Wq.T + bq)  -> [B,T,H,Hd] -> heads
    k = (key_  @ Wk.T + bk)
    v = (value @ Wv.T + bv)
    out = softmax(q k^T * Hd^-0.5) v   (full T x S scores)
    out = out @ Wo.T + bo

Sharding: 8 cores = B(2) x T-quarters(4).  Each core computes all 8 heads
for its 1024 query rows (k/v projections are recomputed per core; no
collectives needed).

Device algorithm (per core, transposed-flash layout):
  - inputs pre-transposed on host: qT [D, Tl], kT [D, S], vT [D, S]
  - projections produce kT_p [D, S] (d on partitions), per-head ZERO-PADDED
    qT tiles (so score matmuls run at K=128 - a K<128 matmul stream never
    warms the PE HAM clock gate: 627ns vs 237ns per matmul, HW-measured),
    and v_aug [S, 8*65]: per head 64 v-columns plus a ones column (ones and
    bv arrive via one bvA tensor_add on the PSUM->SBUF copy)
  - scores transposed: sT[s_tile 128, t 512] = kTp-slice^T @ qTp-slice
  - exp on ScalarE with the 1/sqrt(Hd) scale folded in; 3 PSUM banks per
    activation op to amortize the ~352-cycle ACT fixed overhead
  - PV matmul: lhsT = v_aug slice [s 128, 65], rhs = expT; PSUM accumulates
    [65, 512]; row 64 becomes the softmax denominator (from the ones column)
  - normalize: denominator -> SBUF, reciprocal_approx_fast, gpsimd
    partition_broadcast, one multiply; odd heads land on partitions 64..127
    of the pair-packed raw2 via a small SBUF->SBUF DMA
  - out-proj: K=128 head-PAIR matmuls (4 per output tile), f32 out, bo added
    on host after the gather.
"""

import sys

sys.path.insert(0, "/opt/trn_rl_repo")

import numpy as np

N_CORES = 8
B, T, D, H, HD = 2, 4096, 512, 8, 64
SCALE = HD ** -0.5
TQ = 4                # T-quarters per batch
T_LOC = T // TQ       # 1024 query rows per core
S = T                 # kv sequence length
KC = D // 128         # 4 contraction chunks of 128
NS = S // 128         # 32 s-tiles
VW = H * (HD + 1)     # 520: v_aug width (per head: 64 v cols + ones col)
EXPG = 3              # s-tiles per exp group (3 PSUM banks per ACT op)

_cache = {}


def _build():
    import concourse.bacc as bacc
    import concourse.mybir as mybir
    import concourse.tile as tile

    dt = mybir.dt
    f32, bf16 = dt.float32, dt.bfloat16
    AF = mybir.ActivationFunctionType

    nc = bacc.Bacc("TRN2", target_bir_lowering=False, debug=False,
                   num_devices=N_CORES)

    qT_d = nc.dram_tensor("qT", [D, T_LOC], bf16, kind="ExternalInput").ap()
    kT_d = nc.dram_tensor("kT", [D, S], bf16, kind="ExternalInput").ap()
    vT_d = nc.dram_tensor("vT", [D, S], bf16, kind="ExternalInput").ap()
    wqT_d = nc.dram_tensor("wqT", [D, D], bf16, kind="ExternalInput").ap()
    wkT_d = nc.dram_tensor("wkT", [D, D], bf16, kind="ExternalInput").ap()
    wvA_d = nc.dram_tensor("wvA", [D, VW], bf16, kind="ExternalInput").ap()
    bvA_d = nc.dram_tensor("bvA", [128, VW], bf16, kind="ExternalInput").ap()
    woT_d = nc.dram_tensor("woT", [128, KC * 512], bf16, kind="ExternalInput").ap()
    bq_d = nc.dram_tensor("bq2", [128, KC], f32, kind="ExternalInput").ap()
    bk_d = nc.dram_tensor("bk2", [128, KC], f32, kind="ExternalInput").ap()
    out_d = nc.dram_tensor("out", [T_LOC, D], f32, kind="ExternalOutput").ap()

    with tile.TileContext(nc) as tc:
        with tc.tile_pool(name="persist", bufs=1) as pp, \
             tc.tile_pool(name="inp", bufs=1) as ip, \
             tc.tile_pool(name="ps", bufs=2, space="PSUM") as psp, \
             tc.tile_pool(name="work", bufs=2) as wp:
            # persistent SBUF tensors
            wq_sb = pp.tile([128, KC * 512], bf16, tag="wq")
            wk_sb = pp.tile([128, KC * 512], bf16, tag="wk")
            wv_sb = pp.tile([128, KC * VW], bf16, tag="wv")
            bvA_sb = pp.tile([128, VW], bf16, tag="bvA")
            wo_sb = pp.tile([128, KC * 512], bf16, tag="wo")
            bq_sb = pp.tile([128, KC], f32, tag="bq")
            bk_sb = pp.tile([128, KC], f32, tag="bk")
            # per-head zero-padded qT tiles: head h occupies rows (h%2)*64..+64
            # of tile h, other rows stay zero -> scores run at K=128 (a K=64
            # matmul stream never warms the PE HAM clock gate: 627ns vs 237ns
            # per matmul, HW-measured)
            qTp = pp.tile([128, H * T_LOC], bf16, tag="qTp")
            kTp = pp.tile([128, KC * S], bf16, tag="kTp")
            vA = pp.tile([128, NS * VW], bf16, tag="vA")
            # normalized per-head attention^T, all heads x both t-chunks
            # normalized attention^T, head-PAIR packed: pair m has head 2m at
            # partitions 0..63 and head 2m+1 at 64..127 (odd heads arrive via
            # a small SBUF->SBUF DMA) -> out-proj runs K=128, 4 matmuls/tile
            raw2 = pp.tile([128, KC * T_LOC], bf16, tag="raw2")
            qin = ip.tile([128, KC * T_LOC], bf16, tag="qin")

            # DMA order matters: the first q-proj matmul needs only the ki=0
            # chunks of wq/qin, so those go first; then the rest of q, v, k.
            for ki in range(KC):
                r = slice(ki * 128, (ki + 1) * 128)
                nc.sync.dma_start(wq_sb[:, ki * 512:(ki + 1) * 512], wqT_d[r, :])
                nc.sync.dma_start(qin[:, ki * T_LOC:(ki + 1) * T_LOC], qT_d[r, :])
            nc.sync.dma_start(bq_sb[:, :], bq_d[:, :])
            nc.sync.dma_start(bk_sb[:, :], bk_d[:, :])
            for ki in range(KC):
                r = slice(ki * 128, (ki + 1) * 128)
                nc.sync.dma_start(wv_sb[:, ki * VW:(ki + 1) * VW], wvA_d[r, :])
            nc.sync.dma_start(bvA_sb[:, :], bvA_d[:, :])
            for ki in range(KC):
                r = slice(ki * 128, (ki + 1) * 128)
                nc.sync.dma_start(wk_sb[:, ki * 512:(ki + 1) * 512], wkT_d[r, :])
            nc.sync.dma_start(wo_sb[:, :], woT_d[:, :])

            nc.vector.memset(qTp[:, :], 0.0)

            # ---- qT_p[d, t] (+bq), into zero-padded per-head tiles ----
            # First: a dense K=128 matmul stream warms the PE HAM clock.
            for mi in range(KC):
                for tn in range(T_LOC // 512):
                    psq = psp.tile([128, 512], f32, tag="big", name="psq")
                    for ki in range(KC):
                        nc.tensor.matmul(
                            psq[:, :],
                            lhsT=wq_sb[:, ki * 512 + mi * 128: ki * 512 + (mi + 1) * 128],
                            rhs=qin[:, ki * T_LOC + tn * 512: ki * T_LOC + (tn + 1) * 512],
                            start=(ki == 0), stop=(ki == KC - 1))
                    hA, hB = 2 * mi, 2 * mi + 1
                    nc.vector.tensor_scalar_add(
                        qTp[0:64, hA * T_LOC + tn * 512: hA * T_LOC + (tn + 1) * 512],
                        psq[0:64, :], bq_sb[0:64, mi:mi + 1])
                    nc.vector.tensor_scalar_add(
                        qTp[64:128, hB * T_LOC + tn * 512: hB * T_LOC + (tn + 1) * 512],
                        psq[64:128, :], bq_sb[64:128, mi:mi + 1])

            QS = 1024  # input-streaming quarter size along s

            # ---- kT_p[d, s] and v_aug[s, c], interleaved by s-quarter so
            # attention heads can start as soon as early tiles exist.
            # v_aug columns: c = h*65+j; j<64 v-dims, j=64 the ones column
            # (ones + bv arrive via one bvA tensor_add on the PSUM copy).
            def v_proj(qtr):
                vin_t = ip.tile([128, KC * QS], bf16, tag="vin", bufs=2,
                                name="vin_t")
                for ki in range(KC):
                    nc.sync.dma_start(
                        vin_t[:, ki * QS:(ki + 1) * QS],
                        vT_d[ki * 128:(ki + 1) * 128, qtr * QS:(qtr + 1) * QS])
                for sl in range(QS // 128):
                    si = qtr * (QS // 128) + sl
                    psv = psp.tile([128, VW], f32, tag="big", name="psv")
                    for lo, hi in ((0, 512), (512, VW)):
                        for ki in range(KC):
                            nc.tensor.matmul(
                                psv[:, lo:hi],
                                lhsT=vin_t[:, ki * QS + sl * 128: ki * QS + (sl + 1) * 128],
                                rhs=wv_sb[:, ki * VW + lo: ki * VW + hi],
                                start=(ki == 0), stop=(ki == KC - 1))
                    nc.vector.tensor_add(
                        vA[:, si * VW:(si + 1) * VW], psv[:, :], bvA_sb[:, :])

            def k_proj_qtr(qtr):
                kin_t = ip.tile([128, KC * QS], bf16, tag="kin", bufs=2,
                                name="kin_t")
                for ki in range(KC):
                    nc.sync.dma_start(
                        kin_t[:, ki * QS:(ki + 1) * QS],
                        kT_d[ki * 128:(ki + 1) * 128, qtr * QS:(qtr + 1) * QS])
                for sl in range(QS // 512):
                    sn = qtr * (QS // 512) + sl
                    for mi in range(KC):
                        psk = psp.tile([128, 512], f32, tag="big", name="psk")
                        for ki in range(KC):
                            nc.tensor.matmul(
                                psk[:, :],
                                lhsT=wk_sb[:, ki * 512 + mi * 128: ki * 512 + (mi + 1) * 128],
                                rhs=kin_t[:, ki * QS + sl * 512: ki * QS + (sl + 1) * 512],
                                start=(ki == 0), stop=(ki == KC - 1))
                        nc.vector.tensor_scalar_add(
                            kTp[:, mi * S + sn * 512: mi * S + (sn + 1) * 512],
                            psk[:, :], bk_sb[:, mi:mi + 1])

            def normalize(h, tn, pv):
                den_t = wp.tile([1, 512], f32, tag="den", name="den_t")
                nc.vector.tensor_copy(den_t[:, :], pv[64:65, :])
                recip_t = wp.tile([1, 512], f32, tag="recip", name="recip_t")
                nc.vector.reciprocal_approx_fast(recip_t[:, :], den_t[:, :])
                bc_t = wp.tile([64, 512], f32, tag="bc", name="bc_t")
                nc.gpsimd.partition_broadcast(bc_t[:, :], recip_t[:, :])
                co = (h // 2) * T_LOC + tn * 512
                if h % 2 == 0:
                    nc.vector.tensor_mul(
                        raw2[0:64, co:co + 512], pv[0:64, :], bc_t[:, :])
                else:
                    rtmp = wp.tile([64, 512], bf16, tag="rtmp", name="rtmp")
                    nc.vector.tensor_mul(rtmp[:, :], pv[0:64, :], bc_t[:, :])
                    nc.sync.dma_start(raw2[64:128, co:co + 512], rtmp[:, :])

            def out_proj(tn):
                for tt in range(4):
                    pso = psp.tile([128, 512], f32, tag="big", name="pso")
                    for m in range(KC):
                        co = m * T_LOC + tn * 512 + tt * 128
                        nc.tensor.matmul(
                            pso[:, :],
                            lhsT=raw2[:, co:co + 128],
                            rhs=wo_sb[:, m * 512:(m + 1) * 512],
                            start=(m == 0), stop=(m == KC - 1))
                    out_t = wp.tile([128, 512], f32, tag="out", name="out_t")
                    nc.vector.tensor_copy(out_t[:, :], pso[:, :])
                    nc.sync.dma_start(
                        out_d[tn * 512 + tt * 128: tn * 512 + (tt + 1) * 128, :],
                        out_t[:, :])

            for qtr in range(S // QS):
                k_proj_qtr(qtr)
                v_proj(qtr)
            # out_proj(0) emitted after the first tn=1 attend so its matmuls
            # interleave into the warm stream instead of following a PE idle
            # Flat (stream, si) tile list with exp groups of 3 spanning
            # stream boundaries: the next stream's first scores share a PSUM
            # group with the previous stream's tail, so there is no sc-slot
            # wait at head boundaries; at most 2 streams straddle one group
            # (= the 2 pv slots).  out_proj(0) is emitted after stream (0,1)
            # so its matmuls interleave into the warm stream.
            streams = [(h, 0) for h in range(H)] + [(0, 1)] + \
                      [(h, 1) for h in range(1, H)]
            tiles = [(h, tn, si) for (h, tn) in streams for si in range(NS)]
            pvs = {}
            for g0 in range(0, len(tiles), EXPG):
                grp = tiles[g0:g0 + EXPG]
                sc = psp.tile([128, EXPG * 512], f32, tag="big", name="sc")
                for j, (h, tn, si) in enumerate(grp):
                    mi = h // 2
                    nc.tensor.matmul(
                        sc[:, j * 512:(j + 1) * 512],
                        lhsT=kTp[:, mi * S + si * 128: mi * S + (si + 1) * 128],
                        rhs=qTp[:, h * T_LOC + tn * 512: h * T_LOC + (tn + 1) * 512],
                        start=True, stop=True)
                exp_t = wp.tile([128, EXPG * 512], bf16, tag="exp", bufs=4,
                                name="exp_t")
                nc.scalar.activation(
                    exp_t[:, 0:len(grp) * 512], sc[:, 0:len(grp) * 512],
                    AF.Exp, scale=float(SCALE))
                done = []
                for j, (h, tn, si) in enumerate(grp):
                    if (h, tn) not in pvs:
                        pvs[(h, tn)] = psp.tile([65, 512], f32, tag="pv",
                                                name="pv")
                    nc.tensor.matmul(
                        pvs[(h, tn)][:, :],
                        lhsT=vA[:, si * VW + h * 65: si * VW + (h + 1) * 65],
                        rhs=exp_t[:, j * 512:(j + 1) * 512],
                        start=(si == 0), stop=(si == NS - 1))
                    if si == NS - 1:
                        done.append((h, tn))
                for (h, tn) in done:
                    normalize(h, tn, pvs.pop((h, tn)))
                    if (h, tn) == (0, 1):
                        out_proj(0)
            # keep-warm filler: lowest-priority matmuls the scheduler places
            # into PE idle slots (last normalize chain), so the final
            # out-proj doesn't run at the cooled 1.2GHz HAM clock
            for w in range(16):
                psw = psp.tile([128, 512], f32, tag="big", name="psw")
                nc.tensor.matmul(psw[:, :], lhsT=qTp[:, 0:128],
                                 rhs=qTp[:, 0:512], start=True, stop=True)
            out_proj(1)

    nc.compile()
    return nc


def get_nc():
    if "nc" not in _cache:
        _cache["nc"] = _build()
    return _cache["nc"]


def host_prep(query, key_, value, Wq, bq, Wk, bk, Wv, bv, Wo, bo):
    """Build the 8 per-core input maps (all numpy, bf16 except biases)."""
    import ml_dtypes
    bf16 = ml_dtypes.bfloat16

    def f(x):
        return np.ascontiguousarray(np.asarray(x, dtype=np.float32))

    query, key_, value = f(query), f(key_), f(value)
    Wq, Wk, Wv, Wo = f(Wq), f(Wk), f(Wv), f(Wo)
    bq, bk, bv, bo = f(bq), f(bk), f(bv), f(bo)

    wqT = np.ascontiguousarray(Wq.T).astype(bf16)
    wkT = np.ascontiguousarray(Wk.T).astype(bf16)
    woT = np.concatenate(
        [Wo.T[m * 128:(m + 1) * 128, :] for m in range(KC)], axis=1).astype(bf16)
    # v-projection weights [D, H*(HD+1)] (per-head 64 v-cols + a zero col
    # that bvA turns into the ones column), plus the bias/ones add tile
    wvA = np.zeros((D, VW), dtype=np.float32)
    bvA_row = np.zeros((VW,), dtype=np.float32)
    for h in range(H):
        wvA[:, h * 65: h * 65 + 64] = Wv[h * 64:(h + 1) * 64, :].T
        bvA_row[h * 65: h * 65 + 64] = bv[h * 64:(h + 1) * 64]
        bvA_row[h * 65 + 64] = 1.0
    wvA = wvA.astype(bf16)
    bvA = np.ascontiguousarray(np.broadcast_to(bvA_row, (128, VW))).astype(bf16)
    # per-partition bias layout [128, KC]: col mi = bias[mi*128 : (mi+1)*128]
    bq2 = np.ascontiguousarray(bq.reshape(KC, 128).T)
    bk2 = np.ascontiguousarray(bk.reshape(KC, 128).T)

    in_maps = []
    for c in range(N_CORES):
        b, tq = c // TQ, c % TQ
        qT = np.ascontiguousarray(
            query[b, tq * T_LOC:(tq + 1) * T_LOC, :].T).astype(bf16)
        kT = np.ascontiguousarray(key_[b].T).astype(bf16)
        vT = np.ascontiguousarray(value[b].T).astype(bf16)
        in_maps.append({
            "qT": qT, "kT": kT, "vT": vT,
            "wqT": wqT, "wkT": wkT, "wvA": wvA, "bvA": bvA, "woT": woT,
            "bq2": bq2, "bk2": bk2,
        })
    return in_maps


def gather(results, bo):
    """Assemble full [B, T, D] output from per-core results."""
    out = np.empty((B, T, D), dtype=np.float32)
    for c in range(N_CORES):
        b, tq = c // TQ, c % TQ
        out[b, tq * T_LOC:(tq + 1) * T_LOC, :] = results[c]["out"]
    out += np.asarray(bo, dtype=np.float32)
    return out


def kernel(query, key_, value, Wq, bq, Wk, bk, Wv, bv, Wo, bo):
    from concourse.bass_utils import run_bass_kernel_spmd

    nc = get_nc()
    in_maps = host_prep(query, key_, value, Wq, bq, Wk, bk, Wv, bv, Wo, bo)
    res = run_bass_kernel_spmd(nc, in_maps, core_ids=list(range(N_CORES)))
    _cache["last_result"] = res
    return gather(res.results, bo)

